# revision 4
# baseline (speedup 1.0000x reference)
"""Multi-head attention block (QKV proj + masked softmax + out proj + residual LN)
on 8 Trainium2 NeuronCores.

Sharding: 8 shards = (batch b, query-half); B=4, S=2048. Each core owns one
batch's full K/V and half its queries; no collectives, host concatenates.

Key compaction: the host gathers only the unmasked keys (~1050 of 2048) and
pads to S_KV (multiple of 128). Pad handling: pad key columns of x are 0, so
their V rows are 0, and the denominator "ones" column carries the 0/1 key
mask -- pad keys contribute exactly 0 to numerator AND denominator, so the
exp needs NO mask bias at all.

Per-core strategy (projections bf16, attention scores bf16, PV fp8):
  - scores transposed [k, q], head pair ROW-TILED: head 2p on PE rows 0-63
    (tile (0,0)), head 2p+1 on rows 64-127 (tile (64,0)) -- the two heads'
    score matmuls run CONCURRENTLY in the array (64-deep contraction each).
  - exp on ACT with constant bias -ln(16) (scales softmax numerator AND
    denominator by 1/16 -- cancels exactly; keeps fp8 range safe), written
    as fp8e4 into per-head [128, 2, SQ] chunk-pair interleaved tiles.
  - PV via fp8 DoubleRow: one matmul contracts 256 keys (2 chunks packed
    2/cell), lhsT = [v | mask01] fp8 [128, 2, 65] (80-padded rows), halving
    PV matmul count. Odd trailing chunk uses a plain fp8 matmul.
  - normalization: DVE reciprocal of the sums row + gpsimd partition
    broadcast + DVE multiply (off critical path).
  - y = attn_out @ wo.T, residual via eye-matmul accumulate (+bo folded into
    xq host-side), LayerNorm with rstd = exp(-0.5*log(var+eps)) so ACT stays
    on the natural_log_exp table set (no table switch after the score exps).
  - pipeline: kq(0) emitted first so exp starts ~11us in; V projection and
    kq(p+1) interleaved into the pair loops' PE slack (ACT is the bottleneck
    engine at ~76us of exp work).
"""

import os
import math
import numpy as np
import ml_dtypes

import concourse.bass as bass
import concourse.bacc as bacc
import concourse.tile as tile
import concourse.mybir as mybir
from concourse.bass_utils import run_bass_kernel_spmd

F32 = mybir.dt.float32
BF16 = mybir.dt.bfloat16
F8 = mybir.dt.float8e4
AF = mybir.ActivationFunctionType
ALU = mybir.AluOpType
DR = mybir.MatmulPerfMode.DoubleRow

B, S, D = 4, 2048, 512
H, HD = 8, 64
NCORES = 8
SQ = S // 2          # queries per core
NP = 4               # head pairs
NQT = SQ // 512      # 2 q-tiles of 512
NST = SQ // 128      # 8 output s-tiles
LN16 = math.log(16.0)

_CACHE = {}
LAST_RESULT = None


def _build(ln_trivial, bv_trivial, S_KV, ln_sqrt=False):
    NKT = S_KV // 128
    NKT2 = (NKT + 1) // 2
    nkt_odd = NKT % 2
    key = ("nc", ln_trivial, bv_trivial, S_KV, ln_sqrt)
    if key in _CACHE:
        return _CACHE[key]

    nc = bacc.Bacc("TRN2", target_bir_lowering=False, debug=False, num_devices=NCORES)

    xTk = nc.dram_tensor("xTk", [D, S_KV], BF16, kind="ExternalInput")
    xTq = nc.dram_tensor("xTq", [D, SQ], BF16, kind="ExternalInput")
    xq = nc.dram_tensor("xq", [SQ, D], F32, kind="ExternalInput")
    wqT = nc.dram_tensor("wqT", [D, D], BF16, kind="ExternalInput")
    wkT = nc.dram_tensor("wkT", [D, D], BF16, kind="ExternalInput")
    wvT = nc.dram_tensor("wvT", [D, D], BF16, kind="ExternalInput")
    woT = nc.dram_tensor("woT", [D, D], BF16, kind="ExternalInput")
    bqk = nc.dram_tensor("bqk", [128, 8], F32, kind="ExternalInput")
    if not bv_trivial:
        bv_row = nc.dram_tensor("bv_row", [1, D], BF16, kind="ExternalInput")
    m01 = nc.dram_tensor("m01", [128, 2 * NKT2], F32, kind="ExternalInput")
    temp_b = nc.dram_tensor("temp_b", [128, 1], F32, kind="ExternalInput")
    eye = nc.dram_tensor("eye", [128, 128], F32, kind="ExternalInput")
    if not ln_trivial:
        gamma = nc.dram_tensor("gamma", [1, D], F32, kind="ExternalInput")
        beta = nc.dram_tensor("beta", [1, D], F32, kind="ExternalInput")
    out = nc.dram_tensor("out", [SQ, D], F32, kind="ExternalOutput")

    def dram_bcast(t, p=128):
        a = t.ap()
        return bass.AP(tensor=a.tensor, offset=a.offset, ap=[[0, p]] + list(a.ap)[1:])

    with tile.TileContext(nc) as tc, nc.allow_low_precision(reason="bf16/fp8 matmuls"):
        with tc.tile_pool(name="consts", bufs=1) as consts, \
             tc.tile_pool(name="kqv", bufs=1) as kqv, \
             tc.tile_pool(name="proj", bufs=1) as proj, \
             tc.tile_pool(name="sep", bufs=2) as sep, \
             tc.tile_pool(name="resid", bufs=1) as resid, \
             tc.tile_pool(name="psmm", bufs=2, space="PSUM") as psmm, \
             tc.tile_pool(name="pspv", bufs=2, space="PSUM") as pspv, \
             tc.tile_pool(name="small", bufs=2) as small:

            # ---- constants (small, fast DMAs first) ----
            bqk_t = consts.tile([128, 8], F32, tag="bqk")
            nc.sync.dma_start(out=bqk_t, in_=bqk[:, :])
            m01_t = consts.tile([128, 2 * NKT2], F32, tag="m01")
            nc.sync.dma_start(out=m01_t, in_=m01[:, :])
            tp_t = consts.tile([128, 1], F32, tag="tp")
            nc.sync.dma_start(out=tp_t, in_=temp_b[:, :])
            if not bv_trivial:
                bv_t = consts.tile([1, D], BF16, tag="bv")
                nc.sync.dma_start(out=bv_t, in_=bv_row[:, :])
            if not ln_trivial:
                g_t = consts.tile([128, D], F32, tag="g")
                nc.sync.dma_start(out=g_t, in_=dram_bcast(gamma))
                b_t = consts.tile([128, D], F32, tag="b")
                nc.sync.dma_start(out=b_t, in_=dram_bcast(beta))
            eye_t = consts.tile([128, 128], F32, tag="eye")
            nc.sync.dma_start(out=eye_t, in_=eye[:, :])
            eps_t = consts.tile([128, 1], F32, tag="eps")
            nc.vector.memset(eps_t, 1e-6)
            nln16_t = consts.tile([128, 1], F32, tag="nln16")
            nc.vector.memset(nln16_t, -LN16)
            ones_f = consts.tile([128, 128], F32, tag="onesf")
            nc.vector.memset(ones_f, 1.0)
            ones_b = consts.tile([1, 128], BF16, tag="onesb")
            nc.vector.tensor_copy(out=ones_b, in_=ones_f[0:1, :])

            # ---- input staging: per-chunk DMAs, ordered so kq(0) unblocks
            # first, then V, then the out-proj/residual tail ----
            xtk = proj.tile([128, 4, S_KV], BF16, tag="xtk")
            wk_t = proj.tile([128, 4, D], BF16, tag="wk")
            wq_t = proj.tile([128, 4, D], BF16, tag="wq")
            xtq = proj.tile([128, 4, SQ], BF16, tag="xtq")
            wv_t = proj.tile([128, 4, D], BF16, tag="wv")
            wo_t = consts.tile([128, 4, D], BF16, tag="wo")
            for c in range(4):
                nc.sync.dma_start(out=xtk[:, c, :], in_=xTk[c * 128:(c + 1) * 128, :])
            for c in range(4):
                nc.sync.dma_start(out=wk_t[:, c, :], in_=wkT[c * 128:(c + 1) * 128, :])
            for c in range(4):
                nc.sync.dma_start(out=wq_t[:, c, :], in_=wqT[c * 128:(c + 1) * 128, :])
            for c in range(4):
                nc.sync.dma_start(out=xtq[:, c, :], in_=xTq[c * 128:(c + 1) * 128, :])
            for c in range(4):
                nc.sync.dma_start(out=wv_t[:, c, :], in_=wvT[c * 128:(c + 1) * 128, :])
            for c in range(4):
                nc.sync.dma_start(out=wo_t[:, c, :], in_=woT[c * 128:(c + 1) * 128, :])
            xq_tiles = []
            for st in range(NST):
                xq_t = resid.tile([128, D], F32, tag=f"xq{st}", name=f"xq{st}")
                nc.sync.dma_start(out=xq_t, in_=xq[st * 128:(st + 1) * 128, :])
                xq_tiles.append(xq_t)

            # ---- persistent activations ----
            kT = [kqv.tile([128, S_KV], BF16, tag=f"kT{p}", name=f"kT{p}")
                  for p in range(NP)]
            qT = [kqv.tile([128, SQ], BF16, tag=f"qT{p}", name=f"qT{p}")
                  for p in range(NP)]
            # v per (head, chunk-pair): [v(64) | mask01 | pad to 80] fp8,
            # DoubleRow pair dim in the middle.
            v_all = kqv.tile([128, H, NKT2, 2, 80], F8, tag="vall")
            outn = kqv.tile([128, NP, SQ], BF16, tag="outn")

            # denominator mask column (pads contribute 0): one copy per head
            for h in range(H):
                nc.vector.tensor_copy(out=v_all[:, h, :, :, 64:65], in_=m01_t[:, :])

            # ---- PE warmup during input DMA wait ----
            wu = consts.tile([128, 512], BF16, tag="wu")
            nc.vector.memset(wu, 0.0)
            for i in range(10):
                wps = psmm.tile([128, 2, 512], F32, tag="mm", name=f"warm{i}")
                nc.tensor.matmul(wps[:, 0, :], wu[:, 0:128], wu, start=True, stop=True)
                nc.tensor.matmul(wps[:, 1, :], wu[:, 0:128], wu, start=True, stop=True)

            def emit_kq(p):
                # kT store folds +bk and *temperature (exact when temp=2^-k)
                kv_groups = []
                off0 = 0
                while off0 < S_KV:
                    if S_KV - off0 >= 1024:
                        kv_groups.append((off0, (512, 512))); off0 += 1024
                    else:
                        kv_groups.append((off0, (S_KV - off0,))); off0 += S_KV - off0
                for g0, widths in kv_groups:
                    ps = psmm.tile([128, 2, 512], F32, tag="mm", name=f"psk{p}{g0}")
                    off = g0
                    for j, w in enumerate(widths):
                        for c in range(4):
                            nc.tensor.matmul(
                                ps[:, j, 0:w], wk_t[:, c, p * 128:(p + 1) * 128],
                                xtk[:, c, off:off + w],
                                start=(c == 0), stop=(c == 3))
                        off += w
                    tot = sum(widths)
                    src = ps if len(widths) == 2 else ps[:, 0, 0:tot]
                    nc.vector.tensor_scalar(
                        out=kT[p][:, g0:g0 + tot], in0=src,
                        scalar1=bqk_t[:, 4 + p:5 + p], scalar2=tp_t[:, 0:1],
                        op0=ALU.add, op1=ALU.mult)
                ps = psmm.tile([128, 2, 512], F32, tag="mm", name=f"psq{p}")
                for j in range(2):
                    for c in range(4):
                        nc.tensor.matmul(
                            ps[:, j, :], wq_t[:, c, p * 128:(p + 1) * 128],
                            xtq[:, c, j * 512:(j + 1) * 512],
                            start=(c == 0), stop=(c == 3))
                nc.vector.tensor_scalar_add(
                    out=qT[p][:, :], in0=ps, scalar1=bqk_t[:, p:p + 1])

            def emit_v(t2):
                # V projection for chunk pair t2 (all heads), fp8 store
                for j, t in enumerate(tt for tt in (2 * t2, 2 * t2 + 1) if tt < NKT):
                    ps = psmm.tile([128, 2, 512], F32, tag="mm", name=f"psv{t2}_{j}")
                    for c in range(4):
                        nc.tensor.matmul(
                            ps[:, 0, :], xtk[:, c, t * 128:(t + 1) * 128],
                            wv_t[:, c, :], start=(c == 0),
                            stop=(c == 3 and bv_trivial))
                    if not bv_trivial:
                        nc.tensor.matmul(ps[:, 0, :], ones_b[0:1, :], bv_t,
                                         start=False, stop=True)
                    # one strided copy: [128, (h,64)] -> v_all[:, h, t2, j, 0:64]
                    nc.vector.tensor_copy(
                        out=v_all[:, :, t2, j, 0:64], in_=ps[:, 0, :])

            # insertion schedule: pair_idx -> {kt: [thunks]}
            inserts = {p: {} for p in range(NP)}

            def add_insert(p, kt, fn):
                inserts[p].setdefault(kt, []).append(fn)

            # V(1..NKT2-1) spread through pair 0 (V(t2) needed at kt=2*t2+1)
            for t2 in range(1, NKT2):
                p0kt = max(0, min(2 * t2 - 2, NKT - 1))
                add_insert(0, p0kt, (lambda tt: lambda: emit_v(tt))(t2))
            # kq(p+1) late in pair p
            for p in range(NP - 1):
                add_insert(p, max(0, NKT - 2), (lambda pp: lambda: emit_kq(pp))(p + 1))

            emit_kq(0)
            emit_v(0)

            for p in range(NP):
                se0 = sep.tile([128, 2, SQ], F8, tag="se0", name=f"se0_{p}")
                se1 = sep.tile([128, 2, SQ], F8, tag="se1", name=f"se1_{p}")
                pv0 = pspv.tile([65, 2, 512], F32, tag="pv", name=f"pv{p}_0")
                pv1 = pspv.tile([65, 2, 512], F32, tag="pv", name=f"pv{p}_1")
                ses = (se0, se1)
                pvs = (pv0, pv1)
                for kt in range(NKT):
                    s0 = psmm.tile([128, 2, 512], F32, tag="mm", name=f"s{p}_{kt}_0")
                    s1 = psmm.tile([128, 2, 512], F32, tag="mm", name=f"s{p}_{kt}_1")
                    ksl = slice(kt * 128, (kt + 1) * 128)
                    for qt in range(NQT):
                        qsl = slice(qt * 512, (qt + 1) * 512)
                        nc.tensor.matmul(s0[:, qt, :], kT[p][0:64, ksl],
                                         qT[p][0:64, qsl], start=True, stop=True)
                        nc.tensor.matmul(s1[:, qt, :], kT[p][64:128, ksl],
                                         qT[p][64:128, qsl], start=True, stop=True)
                    nc.scalar.activation(out=se0[:, kt % 2, :], in_=s0,
                                         func=AF.Exp, bias=nln16_t[:, 0:1])
                    nc.scalar.activation(out=se1[:, kt % 2, :], in_=s1,
                                         func=AF.Exp, bias=nln16_t[:, 0:1])
                    for fn in inserts[p].get(kt, ()):
                        fn()
                    if kt % 2 == 1:
                        t2 = kt // 2
                        for h01 in range(2):
                            h = 2 * p + h01
                            for qt in range(NQT):
                                qsl = slice(qt * 512, (qt + 1) * 512)
                                nc.tensor.matmul(
                                    pvs[h01][:, qt, :],
                                    v_all[:, h, t2, :, 0:65],
                                    ses[h01][:, :, qsl],
                                    start=(kt == 1),
                                    stop=(kt == NKT - 1),
                                    perf_mode=DR)
                if nkt_odd:
                    t2 = NKT2 - 1
                    for h01 in range(2):
                        h = 2 * p + h01
                        for qt in range(NQT):
                            qsl = slice(qt * 512, (qt + 1) * 512)
                            nc.tensor.matmul(
                                pvs[h01][:, qt, :],
                                v_all[:, h, t2, 0, 0:65],
                                ses[h01][:, 0, qsl],
                                start=(NKT == 1), stop=True)
                # normalization (hidden under next pair's exps)
                for h01 in range(2):
                    hb = h01 * 64
                    sums = small.tile([1, 1024], F32, tag="sums")
                    nc.vector.tensor_copy(out=sums, in_=pvs[h01][64:65, :, :])
                    rec = small.tile([1, 1024], F32, tag="rec")
                    nc.vector.reciprocal_approx_fast(out=rec, in_=sums)
                    rec_b = small.tile([64, 1024], F32, tag="recb")
                    nc.gpsimd.partition_broadcast(rec_b, rec)
                    nc.vector.tensor_mul(
                        outn[hb:hb + 64, p, :], pvs[h01][0:64, :, :], rec_b)

            # ---- output projection + residual + LayerNorm ----
            for st2 in range(NST // 2):
                yps = psmm.tile([128, 2, 512], F32, tag="mm", name=f"yps{st2}")
                for j in range(2):
                    st = 2 * st2 + j
                    for p in range(NP):
                        nc.tensor.matmul(
                            yps[:, j, :],
                            outn[:, p, st * 128:(st + 1) * 128],
                            wo_t[:, p, :],
                            start=(p == 0), stop=False)
                    nc.tensor.matmul(yps[:, j, :], eye_t, xq_tiles[st],
                                     start=False, stop=True)
                for j in range(2):
                    st = 2 * st2 + j
                    z = yps[:, j, :]
                    stats = small.tile([128, 6], F32, tag="stats")
                    nc.vector.bn_stats(out=stats, in_=z)
                    mv = small.tile([128, 2], F32, tag="mv")
                    nc.vector.bn_aggr(out=mv, in_=stats)
                    rstd = small.tile([128, 1], F32, tag="rstd")
                    if ln_sqrt:
                        std = small.tile([128, 1], F32, tag="std")
                        nc.scalar.activation(out=std, in_=mv[:, 1:2], func=AF.Sqrt,
                                             bias=eps_t[:, 0:1])
                        nc.vector.reciprocal(out=rstd, in_=std)
                    else:
                        # rstd = exp(-0.5*log(var+eps)): stays on the
                        # natural_log_exp ACT table set (no switch after exps)
                        lv = small.tile([128, 1], F32, tag="lv")
                        nc.scalar.activation(out=lv, in_=mv[:, 1:2], func=AF.Ln,
                                             bias=eps_t[:, 0:1])
                        nc.scalar.activation(out=rstd, in_=lv, func=AF.Exp,
                                             scale=-0.5)
                    nb = small.tile([128, 1], F32, tag="nb")
                    nc.vector.tensor_scalar(
                        out=nb, in0=mv[:, 0:1], scalar1=rstd, scalar2=-1.0,
                        op0=ALU.mult, op1=ALU.mult)
                    zn = small.tile([128, D], F32, tag="zn")
                    nc.scalar.activation(out=zn, in_=z, func=AF.Identity,
                                         bias=nb[:, 0:1], scale=rstd[:, 0:1])
                    if ln_trivial:
                        zo = zn
                    else:
                        zg = small.tile([128, D], F32, tag="z")
                        nc.vector.tensor_mul(zg, zn, g_t)
                        zo = small.tile([128, D], F32, tag="zn2")
                        nc.vector.tensor_add(zo, zg, b_t)
                    nc.sync.dma_start(out=out[st * 128:(st + 1) * 128, :], in_=zo)

    nc.compile()
    _CACHE[key] = nc
    return nc


def _prep_in_maps(x, mask, wq, bq, wk, bk, wv, bv, wo, bo, ln_gamma, ln_beta,
                  temperature, ln_trivial, bv_trivial, S_KV):
    f32 = np.float32
    bf16 = ml_dtypes.bfloat16
    x = np.asarray(x, f32)
    mask = np.asarray(mask).astype(bool)
    wqT = np.ascontiguousarray(np.asarray(wq, f32).T).astype(bf16)
    wkT = np.ascontiguousarray(np.asarray(wk, f32).T).astype(bf16)
    wvT = np.ascontiguousarray(np.asarray(wv, f32).T).astype(bf16)
    woT = np.ascontiguousarray(np.asarray(wo, f32).T).astype(bf16)
    bq = np.asarray(bq, f32); bk = np.asarray(bk, f32)
    bv = np.asarray(bv, f32); bo = np.asarray(bo, f32)
    bqk = np.ascontiguousarray(
        np.concatenate([bq.reshape(4, 128).T, bk.reshape(4, 128).T], axis=1)
    ).astype(f32)
    temp_b = np.full((128, 1), np.asarray(temperature, f32).reshape(-1)[0], f32)
    NKT = S_KV // 128
    NKT2 = (NKT + 1) // 2

    in_maps = []
    for m in range(NCORES):
        b, half = m // 2, m % 2
        q0 = half * SQ
        xb = x[b]
        idx = np.where(~mask[b])[0]
        nkv = len(idx)
        assert nkv <= S_KV, f"unmasked keys {nkv} > S_KV={S_KV}"
        xk = np.zeros((S_KV, D), f32)
        xk[:nkv] = xb[idx]
        m01v = np.zeros(2 * NKT2 * 128, f32)
        m01v[:nkv] = 1.0
        im = {
            "eye": np.eye(128, dtype=f32),
            "xTk": np.ascontiguousarray(xk.T).astype(bf16),
            "xTq": np.ascontiguousarray(xb[q0:q0 + SQ].T).astype(bf16),
            "xq": np.ascontiguousarray(xb[q0:q0 + SQ] + bo[None, :]),
            "wqT": wqT, "wkT": wkT, "wvT": wvT, "woT": woT,
            "bqk": bqk,
            "m01": np.ascontiguousarray(m01v.reshape(2 * NKT2, 128).T),
            "temp_b": temp_b,
        }
        if not bv_trivial:
            im["bv_row"] = bv.reshape(1, D).astype(bf16)
        if not ln_trivial:
            im["gamma"] = np.asarray(ln_gamma, f32).reshape(1, D)
            im["beta"] = np.asarray(ln_beta, f32).reshape(1, D)
        in_maps.append(im)
    return in_maps


def kernel(**inputs) -> np.ndarray:
    global LAST_RESULT
    ln_trivial = bool(np.all(np.asarray(inputs["ln_gamma"]) == 1.0)
                      and np.all(np.asarray(inputs["ln_beta"]) == 0.0))
    bv_trivial = bool(np.all(np.asarray(inputs["bv"]) == 0.0))
    maskarr = np.asarray(inputs["mask"]).astype(bool)
    max_unmasked = int((~maskarr).sum(axis=1).max())
    S_KV = max(256, -(-max_unmasked // 128) * 128)
    nc = _build(ln_trivial, bv_trivial, S_KV)
    in_maps = _prep_in_maps(**inputs, ln_trivial=ln_trivial, bv_trivial=bv_trivial,
                            S_KV=S_KV)
    res = run_bass_kernel_spmd(nc, in_maps, core_ids=list(range(NCORES)),
                               trace=bool(os.environ.get("BASS_TRACE")))
    LAST_RESULT = res
    y = np.empty((B, S, D), np.float32)
    for m in range(NCORES):
        b, half = m // 2, m % 2
        y[b, half * SQ:(half + 1) * SQ] = res.results[m]["out"]
    return y


# revision 9
# speedup vs baseline: 1.0821x; 1.0821x over previous
"""Multi-head attention block (QKV proj + masked softmax + out proj + residual LN)
on 8 Trainium2 NeuronCores.

Sharding: 8 shards = (batch b, query-half); B=4, S=2048. Each core owns one
batch's full K/V and half its queries; no collectives, host concatenates.

Key compaction: the host gathers only the unmasked keys (~1050 of 2048) and
pads to S_KV (multiple of 128). Pad handling: pad key columns of x are 0, so
their V rows are 0, and the denominator "ones" column carries the 0/1 key
mask -- pad keys contribute exactly 0 to numerator AND denominator, so the
exp needs NO mask bias at all.

Per-core strategy (projections bf16, attention scores bf16, PV fp8):
  - scores transposed [k, q], head pair ROW-TILED: head 2p on PE rows 0-63
    (tile (0,0)), head 2p+1 on rows 64-127 (tile (64,0)) -- the two heads'
    score matmuls run CONCURRENTLY in the array (64-deep contraction each).
  - exp on ACT with constant bias -ln(16) (scales softmax numerator AND
    denominator by 1/16 -- cancels exactly; keeps fp8 range safe), written
    as fp8e4 into per-head [128, 2, SQ] chunk-pair interleaved tiles.
  - PV via fp8 DoubleRow: one matmul contracts 256 keys (2 chunks packed
    2/cell), lhsT = [v | mask01] fp8 [128, 2, 65] (80-padded rows), halving
    PV matmul count. Odd trailing chunk uses a plain fp8 matmul.
  - normalization: DVE reciprocal of the sums row + gpsimd partition
    broadcast + DVE multiply (off critical path).
  - y = attn_out @ wo.T, residual via eye-matmul accumulate (+bo folded into
    xq host-side), LayerNorm with rstd = exp(-0.5*log(var+eps)) so ACT stays
    on the natural_log_exp table set (no table switch after the score exps).
  - pipeline: kq(0) emitted first so exp starts ~11us in; V projection and
    kq(p+1) interleaved into the pair loops' PE slack (ACT is the bottleneck
    engine at ~76us of exp work).
"""

import os
import math
import numpy as np
import ml_dtypes

import concourse.bass as bass
import concourse.bacc as bacc
import concourse.tile as tile
import concourse.mybir as mybir
from concourse.bass_utils import run_bass_kernel_spmd

F32 = mybir.dt.float32
BF16 = mybir.dt.bfloat16
F8 = mybir.dt.float8e4
AF = mybir.ActivationFunctionType
ALU = mybir.AluOpType
DR = mybir.MatmulPerfMode.DoubleRow

B, S, D = 4, 2048, 512
H, HD = 8, 64
NCORES = 8
SQ = S // 2          # queries per core
NP = 4               # head pairs
NQT = SQ // 512      # 2 q-tiles of 512
NST = SQ // 128      # 8 output s-tiles
LN16 = math.log(16.0)

_CACHE = {}
LAST_RESULT = None


def _build(ln_trivial, bv_trivial, S_KV, ln_sqrt=True):
    NKT = S_KV // 128
    NKT2 = (NKT + 1) // 2
    nkt_odd = NKT % 2
    key = ("nc", ln_trivial, bv_trivial, S_KV, ln_sqrt)
    if key in _CACHE:
        return _CACHE[key]

    nc = bacc.Bacc("TRN2", target_bir_lowering=False, debug=False, num_devices=NCORES)

    xTk = nc.dram_tensor("xTk", [D, S_KV], BF16, kind="ExternalInput")
    xTq = nc.dram_tensor("xTq", [D, SQ], BF16, kind="ExternalInput")
    xq = nc.dram_tensor("xq", [SQ, D], F32, kind="ExternalInput")
    wqT = nc.dram_tensor("wqT", [D, D], BF16, kind="ExternalInput")
    wkT = nc.dram_tensor("wkT", [D, D], BF16, kind="ExternalInput")
    wvT = nc.dram_tensor("wvT", [D, D], BF16, kind="ExternalInput")
    woT = nc.dram_tensor("woT", [D, D], BF16, kind="ExternalInput")
    bqk = nc.dram_tensor("bqk", [128, 8], F32, kind="ExternalInput")
    if not bv_trivial:
        bv_row = nc.dram_tensor("bv_row", [1, D], BF16, kind="ExternalInput")
    m01 = nc.dram_tensor("m01", [128, 2 * NKT2], F32, kind="ExternalInput")
    temp_b = nc.dram_tensor("temp_b", [128, 1], F32, kind="ExternalInput")
    eye = nc.dram_tensor("eye", [128, 128], F32, kind="ExternalInput")
    if not ln_trivial:
        gamma = nc.dram_tensor("gamma", [1, D], F32, kind="ExternalInput")
        beta = nc.dram_tensor("beta", [1, D], F32, kind="ExternalInput")
    out = nc.dram_tensor("out", [SQ, D], F32, kind="ExternalOutput")

    def dram_bcast(t, p=128):
        a = t.ap()
        return bass.AP(tensor=a.tensor, offset=a.offset, ap=[[0, p]] + list(a.ap)[1:])

    with tile.TileContext(nc) as tc, nc.allow_low_precision(reason="bf16/fp8 matmuls"):
        with tc.tile_pool(name="consts", bufs=1) as consts, \
             tc.tile_pool(name="kqv", bufs=1) as kqv, \
             tc.tile_pool(name="proj", bufs=1) as proj, \
             tc.tile_pool(name="sep", bufs=2) as sep, \
             tc.tile_pool(name="resid", bufs=1) as resid, \
             tc.tile_pool(name="psmm", bufs=2, space="PSUM") as psmm, \
             tc.tile_pool(name="pspv", bufs=2, space="PSUM") as pspv, \
             tc.tile_pool(name="small", bufs=2) as small:

            # ---- constants (small, fast DMAs first) ----
            bqk_t = consts.tile([128, 8], F32, tag="bqk")
            nc.sync.dma_start(out=bqk_t, in_=bqk[:, :])
            m01_t = consts.tile([128, 2 * NKT2], F32, tag="m01")
            nc.sync.dma_start(out=m01_t, in_=m01[:, :])
            tp_t = consts.tile([128, 1], F32, tag="tp")
            nc.sync.dma_start(out=tp_t, in_=temp_b[:, :])
            if not bv_trivial:
                bv_t = consts.tile([1, D], BF16, tag="bv")
                nc.sync.dma_start(out=bv_t, in_=bv_row[:, :])
            if not ln_trivial:
                g_t = consts.tile([128, D], F32, tag="g")
                nc.sync.dma_start(out=g_t, in_=dram_bcast(gamma))
                b_t = consts.tile([128, D], F32, tag="b")
                nc.sync.dma_start(out=b_t, in_=dram_bcast(beta))
            eye_t = consts.tile([128, 128], F32, tag="eye")
            nc.sync.dma_start(out=eye_t, in_=eye[:, :])
            eps_t = consts.tile([128, 1], F32, tag="eps")
            nc.vector.memset(eps_t, 1e-6)
            nln16_t = consts.tile([128, 1], F32, tag="nln16")
            nc.vector.memset(nln16_t, -LN16)
            ones_f = consts.tile([128, 128], F32, tag="onesf")
            nc.vector.memset(ones_f, 1.0)
            ones_b = consts.tile([1, 128], BF16, tag="onesb")
            nc.vector.tensor_copy(out=ones_b, in_=ones_f[0:1, :])

            # ---- input staging: per-chunk DMAs, ordered so kq(0) unblocks
            # first, then V, then the out-proj/residual tail ----
            xtk = proj.tile([128, 4, S_KV], BF16, tag="xtk")
            wk_t = proj.tile([128, 4, D], BF16, tag="wk")
            wq_t = proj.tile([128, 4, D], BF16, tag="wq")
            xtq = proj.tile([128, 4, SQ], BF16, tag="xtq")
            wv_t = proj.tile([128, 4, D], BF16, tag="wv")
            wo_t = consts.tile([128, 4, D], BF16, tag="wo")
            for c in range(4):
                nc.sync.dma_start(out=xtk[:, c, :], in_=xTk[c * 128:(c + 1) * 128, :])
            for c in range(4):
                nc.sync.dma_start(out=wk_t[:, c, :], in_=wkT[c * 128:(c + 1) * 128, :])
            for c in range(4):
                nc.sync.dma_start(out=wq_t[:, c, :], in_=wqT[c * 128:(c + 1) * 128, :])
            for c in range(4):
                nc.sync.dma_start(out=xtq[:, c, :], in_=xTq[c * 128:(c + 1) * 128, :])
            for c in range(4):
                nc.sync.dma_start(out=wv_t[:, c, :], in_=wvT[c * 128:(c + 1) * 128, :])
            for c in range(4):
                nc.sync.dma_start(out=wo_t[:, c, :], in_=woT[c * 128:(c + 1) * 128, :])
            xq_tiles = []
            for st in range(NST):
                xq_t = resid.tile([128, D], F32, tag=f"xq{st}", name=f"xq{st}")
                nc.sync.dma_start(out=xq_t, in_=xq[st * 128:(st + 1) * 128, :])
                xq_tiles.append(xq_t)

            # ---- persistent activations ----
            kT = [kqv.tile([128, S_KV], BF16, tag=f"kT{p}", name=f"kT{p}")
                  for p in range(NP)]
            qT = [kqv.tile([128, SQ], BF16, tag=f"qT{p}", name=f"qT{p}")
                  for p in range(NP)]
            # v per (head, chunk-pair): [v(64) | mask01 | pad to 80] fp8,
            # DoubleRow pair dim in the middle.
            v_all = kqv.tile([128, H, NKT2, 2, 80], F8, tag="vall")
            outn = kqv.tile([128, NP, SQ], BF16, tag="outn")

            # denominator mask column (pads contribute 0): one copy per head
            for h in range(H):
                nc.vector.tensor_copy(out=v_all[:, h, :, :, 64:65], in_=m01_t[:, :])

            # ---- PE warmup during input DMA wait ----
            wu = consts.tile([128, 512], BF16, tag="wu")
            nc.vector.memset(wu, 0.0)
            for i in range(5):
                wps = psmm.tile([128, 2, 512], F32, tag="mm", name=f"warm{i}")
                nc.tensor.matmul(wps[:, 0, :], wu[:, 0:128], wu, start=True, stop=True)
                nc.tensor.matmul(wps[:, 1, :], wu[:, 0:128], wu, start=True, stop=True)

            def emit_kq(p):
                # kT store folds +bk and *temperature (exact when temp=2^-k)
                kv_groups = []
                off0 = 0
                while off0 < S_KV:
                    if S_KV - off0 >= 1024:
                        kv_groups.append((off0, (512, 512))); off0 += 1024
                    else:
                        kv_groups.append((off0, (S_KV - off0,))); off0 += S_KV - off0
                for g0, widths in kv_groups:
                    ps = psmm.tile([128, 2, 512], F32, tag="mm", name=f"psk{p}{g0}")
                    off = g0
                    for j, w in enumerate(widths):
                        for c in range(4):
                            nc.tensor.matmul(
                                ps[:, j, 0:w], wk_t[:, c, p * 128:(p + 1) * 128],
                                xtk[:, c, off:off + w],
                                start=(c == 0), stop=(c == 3))
                        off += w
                    tot = sum(widths)
                    src = ps if len(widths) == 2 else ps[:, 0, 0:tot]
                    nc.vector.tensor_scalar(
                        out=kT[p][:, g0:g0 + tot], in0=src,
                        scalar1=bqk_t[:, 4 + p:5 + p], scalar2=tp_t[:, 0:1],
                        op0=ALU.add, op1=ALU.mult)
                ps = psmm.tile([128, 2, 512], F32, tag="mm", name=f"psq{p}")
                for j in range(2):
                    for c in range(4):
                        nc.tensor.matmul(
                            ps[:, j, :], wq_t[:, c, p * 128:(p + 1) * 128],
                            xtq[:, c, j * 512:(j + 1) * 512],
                            start=(c == 0), stop=(c == 3))
                nc.vector.tensor_scalar_add(
                    out=qT[p][:, :], in0=ps, scalar1=bqk_t[:, p:p + 1])

            def emit_v(t2):
                # V projection for chunk pair t2 (all heads), fp8 store
                for j, t in enumerate(tt for tt in (2 * t2, 2 * t2 + 1) if tt < NKT):
                    ps = psmm.tile([128, 2, 512], F32, tag="mm", name=f"psv{t2}_{j}")
                    for c in range(4):
                        nc.tensor.matmul(
                            ps[:, 0, :], xtk[:, c, t * 128:(t + 1) * 128],
                            wv_t[:, c, :], start=(c == 0),
                            stop=(c == 3 and bv_trivial))
                    if not bv_trivial:
                        nc.tensor.matmul(ps[:, 0, :], ones_b[0:1, :], bv_t,
                                         start=False, stop=True)
                    # one strided copy: [128, (h,64)] -> v_all[:, h, t2, j, 0:64]
                    nc.vector.tensor_copy(
                        out=v_all[:, :, t2, j, 0:64], in_=ps[:, 0, :])

            # insertion schedule: pair_idx -> {kt: [thunks]}
            inserts = {p: {} for p in range(NP)}

            def add_insert(p, kt, fn):
                inserts[p].setdefault(kt, []).append(fn)

            # V(1..NKT2-1) spread through pair 0 (V(t2) needed at kt=2*t2+1)
            for t2 in range(1, NKT2):
                p0kt = max(0, min(2 * t2 - 2, NKT - 1))
                add_insert(0, p0kt, (lambda tt: lambda: emit_v(tt))(t2))
            # kq(p+1) early-mid in pair p so its PE burst hides under the
            # ACT exp backlog instead of stalling the pair boundary
            for p in range(NP - 1):
                add_insert(p, min(3, NKT - 1), (lambda pp: lambda: emit_kq(pp))(p + 1))

            emit_kq(0)
            emit_v(0)

            for p in range(NP):
                se0 = sep.tile([128, 2, SQ], F8, tag="se0", name=f"se0_{p}")
                se1 = sep.tile([128, 2, SQ], F8, tag="se1", name=f"se1_{p}")
                pv0 = pspv.tile([65, 2, 512], F32, tag="pv", name=f"pv{p}_0")
                pv1 = pspv.tile([65, 2, 512], F32, tag="pv", name=f"pv{p}_1")
                ses = (se0, se1)
                pvs = (pv0, pv1)
                for kt in range(NKT):
                    s0 = psmm.tile([128, 2, 512], F32, tag="mm", name=f"s{p}_{kt}_0")
                    s1 = psmm.tile([128, 2, 512], F32, tag="mm", name=f"s{p}_{kt}_1")
                    ksl = slice(kt * 128, (kt + 1) * 128)
                    for qt in range(NQT):
                        qsl = slice(qt * 512, (qt + 1) * 512)
                        nc.tensor.matmul(s0[:, qt, :], kT[p][0:64, ksl],
                                         qT[p][0:64, qsl], start=True, stop=True)
                        nc.tensor.matmul(s1[:, qt, :], kT[p][64:128, ksl],
                                         qT[p][64:128, qsl], start=True, stop=True)
                    nc.scalar.activation(out=se0[:, kt % 2, :], in_=s0,
                                         func=AF.Exp, bias=nln16_t[:, 0:1])
                    nc.scalar.activation(out=se1[:, kt % 2, :], in_=s1,
                                         func=AF.Exp, bias=nln16_t[:, 0:1])
                    for fn in inserts[p].get(kt, ()):
                        fn()
                    if kt % 2 == 1:
                        t2 = kt // 2
                        for h01 in range(2):
                            h = 2 * p + h01
                            for qt in range(NQT):
                                qsl = slice(qt * 512, (qt + 1) * 512)
                                nc.tensor.matmul(
                                    pvs[h01][:, qt, :],
                                    v_all[:, h, t2, :, 0:65],
                                    ses[h01][:, :, qsl],
                                    start=(kt == 1),
                                    stop=(kt == NKT - 1),
                                    perf_mode=DR)
                if nkt_odd:
                    t2 = NKT2 - 1
                    for h01 in range(2):
                        h = 2 * p + h01
                        for qt in range(NQT):
                            qsl = slice(qt * 512, (qt + 1) * 512)
                            nc.tensor.matmul(
                                pvs[h01][:, qt, :],
                                v_all[:, h, t2, 0, 0:65],
                                ses[h01][:, 0, qsl],
                                start=(NKT == 1), stop=True)
                # normalization (hidden under next pair's exps)
                for h01 in range(2):
                    hb = h01 * 64
                    sums = small.tile([1, 1024], F32, tag="sums")
                    nc.vector.tensor_copy(out=sums, in_=pvs[h01][64:65, :, :])
                    rec = small.tile([1, 1024], F32, tag="rec")
                    nc.vector.reciprocal_approx_fast(out=rec, in_=sums)
                    rec_b = small.tile([64, 1024], F32, tag="recb")
                    nc.gpsimd.partition_broadcast(rec_b, rec)
                    nc.vector.tensor_mul(
                        outn[hb:hb + 64, p, :], pvs[h01][0:64, :, :], rec_b)

            # ---- output projection + residual + LayerNorm ----
            for st2 in range(NST // 2):
                yps = psmm.tile([128, 2, 512], F32, tag="mm", name=f"yps{st2}")
                for j in range(2):
                    st = 2 * st2 + j
                    for p in range(NP):
                        nc.tensor.matmul(
                            yps[:, j, :],
                            outn[:, p, st * 128:(st + 1) * 128],
                            wo_t[:, p, :],
                            start=(p == 0), stop=False)
                    nc.tensor.matmul(yps[:, j, :], eye_t, xq_tiles[st],
                                     start=False, stop=True)
                for j in range(2):
                    st = 2 * st2 + j
                    z = yps[:, j, :]
                    stats = small.tile([128, 6], F32, tag="stats")
                    nc.vector.bn_stats(out=stats, in_=z)
                    mv = small.tile([128, 2], F32, tag="mv")
                    nc.vector.bn_aggr(out=mv, in_=stats)
                    rstd = small.tile([128, 1], F32, tag="rstd")
                    if ln_sqrt:
                        std = small.tile([128, 1], F32, tag="std")
                        nc.scalar.activation(out=std, in_=mv[:, 1:2], func=AF.Sqrt,
                                             bias=eps_t[:, 0:1])
                        nc.vector.reciprocal(out=rstd, in_=std)
                    else:
                        # rstd = exp(-0.5*log(var+eps)): stays on the
                        # natural_log_exp ACT table set (no switch after exps)
                        lv = small.tile([128, 1], F32, tag="lv")
                        nc.scalar.activation(out=lv, in_=mv[:, 1:2], func=AF.Ln,
                                             bias=eps_t[:, 0:1])
                        nc.scalar.activation(out=rstd, in_=lv, func=AF.Exp,
                                             scale=-0.5)
                    nb = small.tile([128, 1], F32, tag="nb")
                    nc.vector.tensor_scalar(
                        out=nb, in0=mv[:, 0:1], scalar1=rstd, scalar2=-1.0,
                        op0=ALU.mult, op1=ALU.mult)
                    zn = small.tile([128, D], F32, tag="zn")
                    nc.scalar.activation(out=zn, in_=z, func=AF.Identity,
                                         bias=nb[:, 0:1], scale=rstd[:, 0:1])
                    if ln_trivial:
                        zo = zn
                    else:
                        zg = small.tile([128, D], F32, tag="z")
                        nc.vector.tensor_mul(zg, zn, g_t)
                        zo = small.tile([128, D], F32, tag="zn2")
                        nc.vector.tensor_add(zo, zg, b_t)
                    nc.sync.dma_start(out=out[st * 128:(st + 1) * 128, :], in_=zo)

    nc.compile()
    _CACHE[key] = nc
    return nc


def _prep_in_maps(x, mask, wq, bq, wk, bk, wv, bv, wo, bo, ln_gamma, ln_beta,
                  temperature, ln_trivial, bv_trivial, S_KV):
    f32 = np.float32
    bf16 = ml_dtypes.bfloat16
    x = np.asarray(x, f32)
    mask = np.asarray(mask).astype(bool)
    wqT = np.ascontiguousarray(np.asarray(wq, f32).T).astype(bf16)
    wkT = np.ascontiguousarray(np.asarray(wk, f32).T).astype(bf16)
    wvT = np.ascontiguousarray(np.asarray(wv, f32).T).astype(bf16)
    woT = np.ascontiguousarray(np.asarray(wo, f32).T).astype(bf16)
    bq = np.asarray(bq, f32); bk = np.asarray(bk, f32)
    bv = np.asarray(bv, f32); bo = np.asarray(bo, f32)
    bqk = np.ascontiguousarray(
        np.concatenate([bq.reshape(4, 128).T, bk.reshape(4, 128).T], axis=1)
    ).astype(f32)
    temp_b = np.full((128, 1), np.asarray(temperature, f32).reshape(-1)[0], f32)
    NKT = S_KV // 128
    NKT2 = (NKT + 1) // 2

    in_maps = []
    for m in range(NCORES):
        b, half = m // 2, m % 2
        q0 = half * SQ
        xb = x[b]
        idx = np.where(~mask[b])[0]
        nkv = len(idx)
        assert nkv <= S_KV, f"unmasked keys {nkv} > S_KV={S_KV}"
        xk = np.zeros((S_KV, D), f32)
        xk[:nkv] = xb[idx]
        m01v = np.zeros(2 * NKT2 * 128, f32)
        m01v[:nkv] = 1.0
        im = {
            "eye": np.eye(128, dtype=f32),
            "xTk": np.ascontiguousarray(xk.T).astype(bf16),
            "xTq": np.ascontiguousarray(xb[q0:q0 + SQ].T).astype(bf16),
            "xq": np.ascontiguousarray(xb[q0:q0 + SQ] + bo[None, :]),
            "wqT": wqT, "wkT": wkT, "wvT": wvT, "woT": woT,
            "bqk": bqk,
            "m01": np.ascontiguousarray(m01v.reshape(2 * NKT2, 128).T),
            "temp_b": temp_b,
        }
        if not bv_trivial:
            im["bv_row"] = bv.reshape(1, D).astype(bf16)
        if not ln_trivial:
            im["gamma"] = np.asarray(ln_gamma, f32).reshape(1, D)
            im["beta"] = np.asarray(ln_beta, f32).reshape(1, D)
        in_maps.append(im)
    return in_maps


def kernel(**inputs) -> np.ndarray:
    global LAST_RESULT
    ln_trivial = bool(np.all(np.asarray(inputs["ln_gamma"]) == 1.0)
                      and np.all(np.asarray(inputs["ln_beta"]) == 0.0))
    bv_trivial = bool(np.all(np.asarray(inputs["bv"]) == 0.0))
    maskarr = np.asarray(inputs["mask"]).astype(bool)
    max_unmasked = int((~maskarr).sum(axis=1).max())
    S_KV = max(256, -(-max_unmasked // 128) * 128)
    nc = _build(ln_trivial, bv_trivial, S_KV)
    in_maps = _prep_in_maps(**inputs, ln_trivial=ln_trivial, bv_trivial=bv_trivial,
                            S_KV=S_KV)
    res = run_bass_kernel_spmd(nc, in_maps, core_ids=list(range(NCORES)),
                               trace=bool(os.environ.get("BASS_TRACE")))
    LAST_RESULT = res
    y = np.empty((B, S, D), np.float32)
    for m in range(NCORES):
        b, half = m // 2, m % 2
        y[b, half * SQ:(half + 1) * SQ] = res.results[m]["out"]
    return y
